# revision 19
# baseline (speedup 1.0000x reference)
"""Trainium2 Bass kernel for nn_C4ByteTransformer (4-step carry-propagation
softmax table lookup).

Contract: kernel(**inputs) takes FULL inputs (a_emb[4,256], b_emb[4,256],
W1[514,131072], W2_sum[131072,256], W2_carry[131072,2]) and returns the full
[4,256] float32 output.

Math: the tables are the canonical byte-add lookup structure (verified
exactly on host, with a numpy fallback otherwise):
  scores_i[k] = a_emb[i, a] + b_emb[i, b] + carry[c],  k = 512a + 2b + c
  weights = softmax(10*(scores - 2.5));  out_i = weights @ W2_sum;
  carry' = weights @ W2_carry,  W2_sum[k, (a+b+c) & 255] = 1,
  W2_carry[k, a+b+c >= 256] = 1.
Because exp is multiplicative over the separable score, with
EA[a] = exp(10 a_emb[i,a]), EB[b] = exp(10 b_emb[i,b]) and
s = sigmoid(20 carry_1 - 10) (= F1/(F0+F1)):
  out_i[m] = ((1-s) cyc[m] + s cyc[(m-1) mod 256]) / (ZA ZB)
  carry'_1 = (U + V s) / (ZA ZB)
where cyc = 256-point cyclic convolution of EA and EB,
U = sum_{a+b>=256} EA[a]EB[b], V = sum_{a+b=255} EA[a]EB[b].
The 131072-entry table never has to be touched. The host pre-replicates
b_emb into Hankel-window layout (pure input packing); the device exps it
once and runs eight float32r matmuls whose lhsT is a step-masked exp(a)
block (off-step columns are exp(-50) ~ 0), so all eight accumulate into
one [4, 256] PSUM tile, landing step-on-partition with no transpose and
no DRAM round trips. U comes from suffix sums of EB via one constant
triangular matmul; the sums are broadcast onto partitions 0-3 by the
reduction matmul itself, so the 3-op-per-step carry chain and the final
combine run without any cross-partition scatter. One NeuronCore, ~1.1 MB
of input DMA, no collectives, no intermediate DRAM traffic.
"""

import os

import numpy as np

NSTEP = 4
D = 256
NE = 131072

_CACHE = {}

LAST_EXEC_TIME_NS = None


def _build_nc():
    import concourse.bacc as bacc
    import concourse.mybir as mybir
    import concourse.tile as tile

    f32 = mybir.dt.float32
    f32r = mybir.dt.float32r
    mult = mybir.AluOpType.mult
    add = mybir.AluOpType.add
    subtract = mybir.AluOpType.subtract
    Exp = mybir.ActivationFunctionType.Exp
    Sigmoid = mybir.ActivationFunctionType.Sigmoid

    nc = bacc.Bacc("TRN2", target_bir_lowering=False, debug=False,
                   num_devices=1)

    # Inputs (host pre-packed; see _prep_inputs).
    bwin = nc.dram_tensor("bwin", [128, NSTEP, 512], f32,
                          kind="ExternalInput")
    a8 = nc.dram_tensor("a8", [128, 2, NSTEP], f32, kind="ExternalInput")
    a8m = nc.dram_tensor("a8m", [128, 2, NSTEP, NSTEP], f32,
                         kind="ExternalInput")
    b8 = nc.dram_tensor("b8", [128, 2, NSTEP], f32, kind="ExternalInput")
    tri = nc.dram_tensor("tri", [128, 128], f32, kind="ExternalInput")
    onem = nc.dram_tensor("onem", [128, 128], f32, kind="ExternalInput")
    msk = nc.dram_tensor("msk", [NSTEP, 8], f32, kind="ExternalInput")
    out = nc.dram_tensor("out", [NSTEP, D], f32, kind="ExternalOutput")

    with tile.TileContext(nc) as tc:
        with (
            tc.tile_pool(name="sb", bufs=1) as sb,
            tc.tile_pool(name="small", bufs=1) as small,
            tc.tile_pool(name="psA", bufs=1, space="PSUM") as psA,
            tc.tile_pool(name="psC", bufs=1, space="PSUM") as psC,
            tc.tile_pool(name="psD", bufs=1, space="PSUM") as psD,
        ):
            bias0_128 = small.tile([128, 1], f32)
            nc.vector.memset(bias0_128[:], 0.0)
            biasm10_4 = small.tile([NSTEP, 1], f32)
            nc.vector.memset(biasm10_4[:], -10.0)

            # Inputs, spread across queues; bwin halves first (critical path).
            bwin_sb = sb.tile([128, NSTEP, 512], f32)
            nc.sync.dma_start(bwin_sb[:, 0:2, :], bwin[:, 0:2, :])
            nc.gpsimd.dma_start(bwin_sb[:, 2:4, :], bwin[:, 2:4, :])
            b8_sb = sb.tile([128, 2, NSTEP], f32)
            nc.scalar.dma_start(b8_sb[:], b8[:])
            tri_sb = sb.tile([128, 128], f32)
            nc.scalar.dma_start(tri_sb[:], tri[:])
            a8_sb = sb.tile([128, 2, NSTEP], f32)
            nc.scalar.dma_start(a8_sb[:], a8[:])
            a8m_sb = sb.tile([128, 2, NSTEP, NSTEP], f32)
            nc.sync.dma_start(a8m_sb[:], a8m[:])
            one_sb = sb.tile([128, 128], f32)
            nc.gpsimd.dma_start(one_sb[:], onem[:])
            msk_sb = small.tile([NSTEP, 8], f32)
            nc.scalar.dma_start(msk_sb[:], msk[:])

            # exps (eb first: it gates suf -> sums -> carry chain).
            eb = sb.tile([128, 2, NSTEP], f32)
            nc.scalar.activation(eb[:], b8_sb[:], Exp, bias=bias0_128[:],
                                 scale=10.0)
            ea = sb.tile([128, 2, NSTEP], f32)
            nc.scalar.activation(ea[:], a8_sb[:], Exp, bias=bias0_128[:],
                                 scale=10.0)
            eam = sb.tile([128, 2, NSTEP, NSTEP], f32r)
            nc.scalar.activation(eam[:], a8m_sb[:], Exp, bias=bias0_128[:],
                                 scale=10.0)
            # Hankel windows of exp(b), two halves so steps 0-1 start early.
            ewin = sb.tile([128, NSTEP, 512], f32r)
            nc.scalar.activation(ewin[:, 0:2, :], bwin_sb[:, 0:2, :], Exp,
                                 bias=bias0_128[:], scale=10.0)
            nc.scalar.activation(ewin[:, 2:4, :], bwin_sb[:, 2:4, :], Exp,
                                 bias=bias0_128[:], scale=10.0)

            # ---- Suffix sums: suf[p, tc, i] = sum_{b >= 128 tc + p + 1} EB_i[b]
            suf_ps = psA.tile([128, 2, NSTEP], f32)
            nc.tensor.matmul(suf_ps[:, 0, :], lhsT=tri_sb[:], rhs=eb[:, 0, :],
                             start=True, stop=False)
            nc.tensor.matmul(suf_ps[:, 0, :], lhsT=one_sb[:], rhs=eb[:, 1, :],
                             start=False, stop=True)
            nc.tensor.matmul(suf_ps[:, 1, :], lhsT=tri_sb[:], rhs=eb[:, 1, :],
                             start=True, stop=True)
            suf_sb = sb.tile([128, 2, NSTEP], f32)
            nc.vector.tensor_copy(out=suf_sb[:], in_=suf_ps[:])

            # ---- U/V element products; partition-reduce via ones matmul.
            # lhsT = 4 identical ones columns -> sums broadcast to parts 0-3.
            scr = sb.tile([128, 4, NSTEP], f32)
            nc.vector.tensor_tensor(out=scr[:, 0, :], in0=ea[:, 0, :],
                                    in1=suf_sb[:, 1, :], op=mult)
            nc.vector.tensor_tensor(out=scr[:, 1, :], in0=ea[:, 1, :],
                                    in1=suf_sb[:, 0, :], op=mult)
            nc.vector.tensor_tensor(out=scr[:, 2, :], in0=ea[:, 0, :],
                                    in1=eb[:, 1, :], op=mult)
            nc.vector.tensor_tensor(out=scr[:, 3, :], in0=ea[:, 1, :],
                                    in1=eb[:, 0, :], op=mult)

            red_ps = psC.tile([NSTEP, 8, NSTEP], f32)
            ones4 = one_sb[:, 0:4]
            nc.tensor.matmul(red_ps[:, 0:4, :].opt(), lhsT=ones4,
                             rhs=scr[:].opt(), start=True, stop=True)
            nc.tensor.matmul(red_ps[:, 4:6, :].opt(), lhsT=ones4,
                             rhs=ea[:].opt(), start=True, stop=True)
            nc.tensor.matmul(red_ps[:, 6:8, :].opt(), lhsT=ones4,
                             rhs=eb[:].opt(), start=True, stop=True)
            red_sb = small.tile([NSTEP, 8, NSTEP], f32)
            nc.vector.tensor_copy(out=red_sb[:], in_=red_ps[:])

            # sums[p, k, i]: k = 0:U, 1:V, 2:ZA, 3:ZB (fold the ah pairs)
            sums = small.tile([NSTEP, 4, NSTEP], f32)
            for k in range(4):
                nc.vector.tensor_tensor(
                    out=sums[:, k:k + 1, :],
                    in0=red_sb[:, 2 * k, :].unsqueeze(1),
                    in1=red_sb[:, 2 * k + 1, :].unsqueeze(1), op=add)
            zab = small.tile([NSTEP, NSTEP], f32)
            nc.vector.tensor_tensor(out=zab[:], in0=sums[:, 2, :],
                                    in1=sums[:, 3, :], op=mult)
            zbi = small.tile([NSTEP, NSTEP], f32)
            nc.vector.reciprocal(zbi[:], zab[:])

            # ---- Carry chain on partitions 0-3 (sigmoid form) ----
            cc = small.tile([NSTEP, 1], f32)
            nc.vector.memset(cc[:], 0.0)  # carry_1 = 0
            ss = small.tile([NSTEP, 1], f32)
            tt = small.tile([NSTEP, 1], f32)
            scal = small.tile([NSTEP, 8], f32)  # (beta_i, alpha_i) pairs
            for i in range(NSTEP):
                # s = sigmoid(20 c1 - 10) = F1/(F0+F1)
                nc.scalar.activation(ss[:], cc[:], Sigmoid,
                                     bias=biasm10_4[:], scale=20.0)
                beta = scal[:, 2 * i:2 * i + 1]
                nc.vector.tensor_tensor(out=beta, in0=ss[:],
                                        in1=zbi[:, i:i + 1], op=mult)
                nc.vector.tensor_tensor(out=scal[:, 2 * i + 1:2 * i + 2],
                                        in0=zbi[:, i:i + 1], in1=beta,
                                        op=subtract)
                if i + 1 < NSTEP:
                    # c1' = (V s + U) / ZAB
                    nc.vector.scalar_tensor_tensor(
                        out=tt[:], in0=sums[:, 1, i:i + 1], scalar=ss[:],
                        in1=sums[:, 0, i:i + 1], op0=mult, op1=add)
                    nc.vector.tensor_tensor(out=cc[:], in0=tt[:],
                                            in1=zbi[:, i:i + 1], op=mult)

            # lsb[i, 0:2] = (beta_i, alpha_i) selected via the one-hot mask.
            tmp8 = small.tile([NSTEP, 8], f32)
            nc.vector.tensor_tensor(out=tmp8[:], in0=scal[:], in1=msk_sb[:],
                                    op=mult)
            lsb = small.tile([NSTEP, 2], f32)
            nc.vector.tensor_tensor(out=lsb[:], in0=tmp8[:, 0:2],
                                    in1=tmp8[:, 2:4], op=add)
            nc.vector.tensor_tensor(out=tmp8[:, 4:6], in0=tmp8[:, 4:6],
                                    in1=tmp8[:, 6:8], op=add)
            nc.vector.tensor_tensor(out=lsb[:], in0=lsb[:],
                                    in1=tmp8[:, 4:6], op=add)

            # ---- Convolutions: 8 matmuls accumulate into prt[i, m] ----
            # lhsT = masked exp(a) block (off-step columns ~ exp(-50));
            # rhs ah=0: [V0|V1], ah=1: [V1|V0]. float32r single-pass PE mode.
            prt = psD.tile([NSTEP, 256], f32)
            for i in range(NSTEP):
                for ah in range(2):
                    nc.tensor.matmul(
                        prt[:],
                        lhsT=eam[:, ah, i, :],
                        rhs=ewin[:, i, 128 * ah:128 * ah + 256],
                        start=(i == 0 and ah == 0),
                        stop=(i == NSTEP - 1 and ah == 1),
                    )

            # out[i, m] = alpha_i cyc[m] + beta_i cyc[m-1], straight off PSUM.
            comb = small.tile([NSTEP, D], f32)
            nc.vector.tensor_scalar(out=comb[:], in0=prt[:],
                                    scalar1=lsb[:, 1:2], scalar2=None,
                                    op0=mult)
            nc.vector.scalar_tensor_tensor(out=comb[:, 1:256],
                                           in0=prt[:, 0:255],
                                           scalar=lsb[:, 0:1],
                                           in1=comb[:, 1:256],
                                           op0=mult, op1=add)
            nc.vector.scalar_tensor_tensor(out=comb[:, 0:1],
                                           in0=prt[:, 255:256],
                                           scalar=lsb[:, 0:1],
                                           in1=comb[:, 0:1],
                                           op0=mult, op1=add)
            nc.sync.dma_start(out[:], comb[:])

    nc.compile()
    return nc


def _structure_ok(W1, W2_sum, W2_carry):
    """Exact check that the tables are the canonical byte-add structure."""
    k = np.arange(NE)
    a = k >> 9
    b = (k >> 1) & 255
    c = k & 1
    total = a + b + c
    if W1.shape != (514, NE) or W2_sum.shape != (NE, D):
        return False
    if W2_carry.shape != (NE, 2):
        return False
    if not (W1[a, k] == 1.0).all():
        return False
    if not (W1[256 + b, k] == 1.0).all():
        return False
    if not (W1[512 + c, k] == 1.0).all():
        return False
    if np.abs(W1).sum(dtype=np.float64) != 3.0 * NE:
        return False
    if not (W2_sum[k, total & 255] == 1.0).all():
        return False
    if np.abs(W2_sum).sum(dtype=np.float64) != float(NE):
        return False
    if not (W2_carry[k, (total >= 256).astype(np.int64)] == 1.0).all():
        return False
    if np.abs(W2_carry).sum(dtype=np.float64) != float(NE):
        return False
    return True


def _numpy_fallback(a_emb, b_emb, W1, W2_sum, W2_carry):
    carry = np.zeros(2, dtype=np.float64)
    carry[0] = 1.0
    outs = []
    W1 = W1.astype(np.float64)
    for i in range(NSTEP):
        x = np.concatenate([a_emb[i], b_emb[i], carry]).astype(np.float64)
        scores = x @ W1
        z = (scores - 2.5) * 10.0
        z -= z.max()
        w = np.exp(z)
        w /= w.sum()
        outs.append(w @ W2_sum.astype(np.float64))
        carry = w @ W2_carry.astype(np.float64)
    return np.stack(outs).astype(np.float32)


def _prep_inputs(a_emb, b_emb):
    p = np.arange(128)
    # bwin[j, i, x] = b_emb[i, (j + x + 129) mod 256]
    b_ext = np.take(b_emb, (np.arange(639) + 129) % 256, axis=1)
    bwin = np.ascontiguousarray(
        np.lib.stride_tricks.sliding_window_view(b_ext, 512, axis=1)
        .transpose(1, 0, 2)
    ).astype(np.float32)
    # a8[p, ah, i] = a_emb[i, 128 ah + 127 - p]
    a_r = a_emb[:, ::-1]
    a8 = np.ascontiguousarray(
        a_r.reshape(NSTEP, 2, 128)[:, ::-1, :].transpose(2, 1, 0)
    ).astype(np.float32)
    # a8m: step-masked copy (off-step columns -5 -> exp(10x) ~ 2e-22)
    a8m = np.full((128, 2, NSTEP, NSTEP), -5.0, dtype=np.float32)
    for i in range(NSTEP):
        a8m[:, :, i, i] = a8[:, :, i]
    # b8[p, bh, i] = b_emb[i, 128 bh + p]
    b8 = np.ascontiguousarray(
        b_emb.reshape(NSTEP, 2, 128).transpose(2, 1, 0)
    ).astype(np.float32)
    tri = (p[:, None] >= p[None, :] + 1).astype(np.float32)
    onem = np.ones((128, 128), dtype=np.float32)
    msk = (np.arange(8)[None, :] // 2 == np.arange(NSTEP)[:, None]).astype(
        np.float32
    )
    return {"bwin": bwin, "a8": a8, "a8m": a8m, "b8": b8, "tri": tri,
            "onem": onem, "msk": msk}


def kernel(a_emb, b_emb, W1, W2_sum, W2_carry):
    global LAST_EXEC_TIME_NS
    a_emb = np.asarray(a_emb, dtype=np.float32)
    b_emb = np.asarray(b_emb, dtype=np.float32)
    W1 = np.asarray(W1, dtype=np.float32)
    W2_sum = np.asarray(W2_sum, dtype=np.float32)
    W2_carry = np.asarray(W2_carry, dtype=np.float32)

    if not _structure_ok(W1, W2_sum, W2_carry):
        return _numpy_fallback(a_emb, b_emb, W1, W2_sum, W2_carry)

    from concourse.bass_utils import run_bass_kernel_spmd

    if "nc" not in _CACHE:
        _CACHE["nc"] = _build_nc()
    nc = _CACHE["nc"]

    in_map = _prep_inputs(a_emb, b_emb)
    trace = os.environ.get("KERNEL_TRACE", "") == "1"
    res = run_bass_kernel_spmd(nc, [in_map], [0], trace=trace)
    LAST_EXEC_TIME_NS = res.exec_time_ns
    return np.asarray(res.results[0]["out"], dtype=np.float32)


# revision 23
# speedup vs baseline: 1.1087x; 1.1087x over previous
"""Trainium2 Bass kernel for nn_C4ByteTransformer (4-step carry-propagation
softmax table lookup).

Contract: kernel(**inputs) takes FULL inputs (a_emb[4,256], b_emb[4,256],
W1[514,131072], W2_sum[131072,256], W2_carry[131072,2]) and returns the full
[4,256] float32 output.

Math: the tables are the canonical byte-add lookup structure (verified
exactly on host, with a numpy fallback otherwise):
  scores_i[k] = a_emb[i, a] + b_emb[i, b] + carry[c],  k = 512a + 2b + c
  weights = softmax(10*(scores - 2.5));  out_i = weights @ W2_sum;
  carry' = weights @ W2_carry,  W2_sum[k, (a+b+c) & 255] = 1,
  W2_carry[k, a+b+c >= 256] = 1.
Because exp is multiplicative over the separable score, with
EA[a] = exp(10 a_emb[i,a]), EB[b] = exp(10 b_emb[i,b]) and
s = sigmoid(20 carry_1 - 10) (= F1/(F0+F1)):
  out_i[m] = ((1-s) cyc[m] + s cyc[(m-1) mod 256]) / (ZA ZB)
  carry'_1 = (U + V s) / (ZA ZB)
where cyc = 256-point cyclic convolution of EA and EB,
U = sum_{a+b>=256} EA[a]EB[b], V = sum_{a+b=255} EA[a]EB[b].
The 131072-entry table never has to be touched. The host pre-replicates
b_emb into Hankel-window layout (pure input packing); the device exps it
once and runs eight float32r matmuls whose lhsT is a step-masked exp(a)
block (off-step columns are exp(-50) ~ 0), so all eight accumulate into
one [4, 256] PSUM tile, landing step-on-partition with no transpose and
no DRAM round trips. U comes from suffix sums of EB via one constant
triangular matmul; the sums are broadcast onto partitions 0-3 by the
reduction matmul itself, so the 3-op-per-step carry chain and the final
combine run without any cross-partition scatter. One NeuronCore, ~1.1 MB
of input DMA, no collectives, no intermediate DRAM traffic.
"""

import os

import numpy as np

NSTEP = 4
D = 256
NE = 131072

_CACHE = {}

LAST_EXEC_TIME_NS = None


def _build_nc():
    import concourse.bacc as bacc
    import concourse.mybir as mybir
    import concourse.tile as tile

    f32 = mybir.dt.float32
    f32r = mybir.dt.float32r
    mult = mybir.AluOpType.mult
    add = mybir.AluOpType.add
    subtract = mybir.AluOpType.subtract
    Exp = mybir.ActivationFunctionType.Exp

    nc = bacc.Bacc("TRN2", target_bir_lowering=False, debug=False,
                   num_devices=1)

    # Inputs (host pre-packed; see _prep_inputs).
    bwin = nc.dram_tensor("bwin", [128, NSTEP, 512], f32,
                          kind="ExternalInput")
    a8 = nc.dram_tensor("a8", [128, 2, NSTEP], f32, kind="ExternalInput")
    a8m = nc.dram_tensor("a8m", [128, 2, NSTEP, NSTEP], f32,
                         kind="ExternalInput")
    b8 = nc.dram_tensor("b8", [128, 2, NSTEP], f32, kind="ExternalInput")
    tri = nc.dram_tensor("tri", [128, 128], f32, kind="ExternalInput")
    msk = nc.dram_tensor("msk", [NSTEP, 8], f32, kind="ExternalInput")
    out = nc.dram_tensor("out", [NSTEP, D], f32, kind="ExternalOutput")

    S0 = float(1.0 / (1.0 + np.exp(10.0)))  # sigmoid(-10): step-0 carry wt

    with tile.TileContext(nc) as tc:
        with (
            tc.tile_pool(name="sb", bufs=1) as sb,
            tc.tile_pool(name="small", bufs=1) as small,
            tc.tile_pool(name="psA", bufs=1, space="PSUM") as psA,
            tc.tile_pool(name="psC", bufs=1, space="PSUM") as psC,
            tc.tile_pool(name="psD", bufs=1, space="PSUM") as psD,
        ):
            bias0_128 = small.tile([128, 1], f32)
            nc.vector.memset(bias0_128[:], 0.0)
            bias10_4 = small.tile([NSTEP, 1], f32)
            nc.vector.memset(bias10_4[:], 10.0)
            one_sb = sb.tile([128, 128], f32)
            nc.vector.memset(one_sb[:], 1.0)
            one_4 = small.tile([NSTEP, 1], f32)
            nc.vector.memset(one_4[:], 1.0)
            s0_4 = small.tile([NSTEP, 1], f32)
            nc.vector.memset(s0_4[:], S0)

            # Inputs. Bulk (bwin, per-step quarters) on the gpsimd queue --
            # the sync queue moves bulk ~4x slower. Scalar issues tri/msk
            # before its ACT stream begins; sync takes the small a/b packs.
            bwin_sb = sb.tile([128, NSTEP, 512], f32)
            for i in range(NSTEP):
                nc.gpsimd.dma_start(bwin_sb[:, i:i + 1, :], bwin[:, i:i + 1, :])
            tri_sb = sb.tile([128, 128], f32)
            nc.scalar.dma_start(tri_sb[:], tri[:])
            msk_sb = small.tile([NSTEP, 8], f32)
            nc.scalar.dma_start(msk_sb[:], msk[:])
            b8_sb = sb.tile([128, 2, NSTEP], f32)
            nc.sync.dma_start(b8_sb[:], b8[:])
            a8_sb = sb.tile([128, 2, NSTEP], f32)
            nc.sync.dma_start(a8_sb[:], a8[:])
            a8m_sb = sb.tile([128, 2, NSTEP, NSTEP], f32)
            nc.sync.dma_start(a8m_sb[:], a8m[:])

            # exps (eb first: it gates suf -> sums -> carry chain).
            eb = sb.tile([128, 2, NSTEP], f32)
            nc.scalar.activation(eb[:], b8_sb[:], Exp, bias=bias0_128[:],
                                 scale=10.0)
            ea = sb.tile([128, 2, NSTEP], f32)
            nc.scalar.activation(ea[:], a8_sb[:], Exp, bias=bias0_128[:],
                                 scale=10.0)
            eam = sb.tile([128, 2, NSTEP, NSTEP], f32r)
            nc.scalar.activation(eam[:], a8m_sb[:], Exp, bias=bias0_128[:],
                                 scale=10.0)
            # Hankel windows of exp(b), per step so conv matmuls start early.
            ewin = sb.tile([128, NSTEP, 512], f32r)
            for i in range(NSTEP):
                nc.scalar.activation(ewin[:, i:i + 1, :], bwin_sb[:, i:i + 1, :],
                                     Exp, bias=bias0_128[:], scale=10.0)

            # ---- Suffix sums: suf[p, tc, i] = sum_{b >= 128 tc + p + 1} EB_i[b]
            suf_ps = psA.tile([128, 2, NSTEP], f32)
            nc.tensor.matmul(suf_ps[:, 0, :], lhsT=tri_sb[:], rhs=eb[:, 0, :],
                             start=True, stop=False)
            nc.tensor.matmul(suf_ps[:, 0, :], lhsT=one_sb[:], rhs=eb[:, 1, :],
                             start=False, stop=True)
            nc.tensor.matmul(suf_ps[:, 1, :], lhsT=tri_sb[:], rhs=eb[:, 1, :],
                             start=True, stop=True)
            suf_sb = sb.tile([128, 2, NSTEP], f32)
            nc.vector.tensor_copy(out=suf_sb[:], in_=suf_ps[:])

            # ---- U/V element products; partition-reduce via ones matmul.
            # lhsT = 4 identical ones columns -> sums broadcast to parts 0-3.
            scr = sb.tile([128, 4, NSTEP], f32)
            nc.vector.tensor_tensor(out=scr[:, 0, :], in0=ea[:, 0, :],
                                    in1=suf_sb[:, 1, :], op=mult)
            nc.vector.tensor_tensor(out=scr[:, 1, :], in0=ea[:, 1, :],
                                    in1=suf_sb[:, 0, :], op=mult)
            nc.vector.tensor_tensor(out=scr[:, 2, :], in0=ea[:, 0, :],
                                    in1=eb[:, 1, :], op=mult)
            nc.vector.tensor_tensor(out=scr[:, 3, :], in0=ea[:, 1, :],
                                    in1=eb[:, 0, :], op=mult)

            red_ps = psC.tile([NSTEP, 8, NSTEP], f32)
            ones4 = one_sb[:, 0:4]
            nc.tensor.matmul(red_ps[:, 0:4, :].opt(), lhsT=ones4,
                             rhs=scr[:].opt(), start=True, stop=True)
            nc.tensor.matmul(red_ps[:, 4:6, :].opt(), lhsT=ones4,
                             rhs=ea[:].opt(), start=True, stop=True)
            nc.tensor.matmul(red_ps[:, 6:8, :].opt(), lhsT=ones4,
                             rhs=eb[:].opt(), start=True, stop=True)
            red_sb = small.tile([NSTEP, 8, NSTEP], f32)
            nc.vector.tensor_copy(out=red_sb[:], in_=red_ps[:])

            # sums[p, k, i]: k = 0:U, 1:V, 2:ZA, 3:ZB (fold the ah pairs)
            sums = small.tile([NSTEP, 4, NSTEP], f32)
            for k in range(4):
                nc.vector.tensor_tensor(
                    out=sums[:, k:k + 1, :],
                    in0=red_sb[:, 2 * k, :].unsqueeze(1),
                    in1=red_sb[:, 2 * k + 1, :].unsqueeze(1), op=add)
            zab = small.tile([NSTEP, NSTEP], f32)
            nc.vector.tensor_tensor(out=zab[:], in0=sums[:, 2, :],
                                    in1=sums[:, 3, :], op=mult)
            zbi = small.tile([NSTEP, NSTEP], f32)
            nc.vector.reciprocal(zbi[:], zab[:])

            # ---- Carry chain on partitions 0-3 ----
            # s_i = F1/(F0+F1) = 1/(1 + exp(10 - 20 c1)); step 0 is the
            # constant sigmoid(-10). Exp-only so the ACT table never swaps.
            cc = small.tile([NSTEP, 1], f32)
            rr = small.tile([NSTEP, 1], f32)
            ss = small.tile([NSTEP, 1], f32)
            tt = small.tile([NSTEP, 1], f32)
            scal = small.tile([NSTEP, 8], f32)  # (beta_i, alpha_i) pairs
            for i in range(NSTEP):
                if i == 0:
                    ss_i = s0_4[:]
                else:
                    # r = exp(10 - 20 c1); s = 1/(1 + r)
                    nc.scalar.activation(rr[:], cc[:], Exp, bias=bias10_4[:],
                                         scale=-20.0)
                    nc.vector.tensor_tensor(out=tt[:], in0=rr[:],
                                            in1=one_4[:], op=add)
                    nc.vector.reciprocal(ss[:], tt[:])
                    ss_i = ss[:]
                beta = scal[:, 2 * i:2 * i + 1]
                nc.vector.tensor_tensor(out=beta, in0=ss_i,
                                        in1=zbi[:, i:i + 1], op=mult)
                nc.vector.tensor_tensor(out=scal[:, 2 * i + 1:2 * i + 2],
                                        in0=zbi[:, i:i + 1], in1=beta,
                                        op=subtract)
                if i + 1 < NSTEP:
                    # c1' = (V s + U) / ZAB
                    nc.vector.scalar_tensor_tensor(
                        out=tt[:], in0=sums[:, 1, i:i + 1], scalar=ss_i,
                        in1=sums[:, 0, i:i + 1], op0=mult, op1=add)
                    nc.vector.tensor_tensor(out=cc[:], in0=tt[:],
                                            in1=zbi[:, i:i + 1], op=mult)

            # lsb[i, 0:2] = (beta_i, alpha_i) selected via the one-hot mask.
            tmp8 = small.tile([NSTEP, 8], f32)
            nc.vector.tensor_tensor(out=tmp8[:], in0=scal[:], in1=msk_sb[:],
                                    op=mult)
            lsb = small.tile([NSTEP, 2], f32)
            nc.vector.tensor_tensor(out=lsb[:], in0=tmp8[:, 0:2],
                                    in1=tmp8[:, 2:4], op=add)
            nc.vector.tensor_tensor(out=tmp8[:, 4:6], in0=tmp8[:, 4:6],
                                    in1=tmp8[:, 6:8], op=add)
            nc.vector.tensor_tensor(out=lsb[:], in0=lsb[:],
                                    in1=tmp8[:, 4:6], op=add)

            # ---- Convolutions: 8 matmuls accumulate into prt[i, m] ----
            # lhsT = masked exp(a) block (off-step columns ~ exp(-50));
            # rhs ah=0: [V0|V1], ah=1: [V1|V0]. float32r single-pass PE mode.
            prt = psD.tile([NSTEP, 256], f32)
            for i in range(NSTEP):
                for ah in range(2):
                    nc.tensor.matmul(
                        prt[:],
                        lhsT=eam[:, ah, i, :],
                        rhs=ewin[:, i, 128 * ah:128 * ah + 256],
                        start=(i == 0 and ah == 0),
                        stop=(i == NSTEP - 1 and ah == 1),
                    )

            # out[i, m] = alpha_i cyc[m] + beta_i cyc[m-1], straight off PSUM.
            comb = small.tile([NSTEP, D], f32)
            nc.vector.tensor_scalar(out=comb[:], in0=prt[:],
                                    scalar1=lsb[:, 1:2], scalar2=None,
                                    op0=mult)
            nc.vector.scalar_tensor_tensor(out=comb[:, 1:256],
                                           in0=prt[:, 0:255],
                                           scalar=lsb[:, 0:1],
                                           in1=comb[:, 1:256],
                                           op0=mult, op1=add)
            nc.vector.scalar_tensor_tensor(out=comb[:, 0:1],
                                           in0=prt[:, 255:256],
                                           scalar=lsb[:, 0:1],
                                           in1=comb[:, 0:1],
                                           op0=mult, op1=add)
            nc.sync.dma_start(out[:], comb[:])

    nc.compile()
    return nc


def _structure_ok(W1, W2_sum, W2_carry):
    """Exact check that the tables are the canonical byte-add structure."""
    k = np.arange(NE)
    a = k >> 9
    b = (k >> 1) & 255
    c = k & 1
    total = a + b + c
    if W1.shape != (514, NE) or W2_sum.shape != (NE, D):
        return False
    if W2_carry.shape != (NE, 2):
        return False
    if not (W1[a, k] == 1.0).all():
        return False
    if not (W1[256 + b, k] == 1.0).all():
        return False
    if not (W1[512 + c, k] == 1.0).all():
        return False
    if np.abs(W1).sum(dtype=np.float64) != 3.0 * NE:
        return False
    if not (W2_sum[k, total & 255] == 1.0).all():
        return False
    if np.abs(W2_sum).sum(dtype=np.float64) != float(NE):
        return False
    if not (W2_carry[k, (total >= 256).astype(np.int64)] == 1.0).all():
        return False
    if np.abs(W2_carry).sum(dtype=np.float64) != float(NE):
        return False
    return True


def _numpy_fallback(a_emb, b_emb, W1, W2_sum, W2_carry):
    carry = np.zeros(2, dtype=np.float64)
    carry[0] = 1.0
    outs = []
    W1 = W1.astype(np.float64)
    for i in range(NSTEP):
        x = np.concatenate([a_emb[i], b_emb[i], carry]).astype(np.float64)
        scores = x @ W1
        z = (scores - 2.5) * 10.0
        z -= z.max()
        w = np.exp(z)
        w /= w.sum()
        outs.append(w @ W2_sum.astype(np.float64))
        carry = w @ W2_carry.astype(np.float64)
    return np.stack(outs).astype(np.float32)


def _prep_inputs(a_emb, b_emb):
    p = np.arange(128)
    # bwin[j, i, x] = b_emb[i, (j + x + 129) mod 256]
    b_ext = np.take(b_emb, (np.arange(639) + 129) % 256, axis=1)
    bwin = np.ascontiguousarray(
        np.lib.stride_tricks.sliding_window_view(b_ext, 512, axis=1)
        .transpose(1, 0, 2)
    ).astype(np.float32)
    # a8[p, ah, i] = a_emb[i, 128 ah + 127 - p]
    a_r = a_emb[:, ::-1]
    a8 = np.ascontiguousarray(
        a_r.reshape(NSTEP, 2, 128)[:, ::-1, :].transpose(2, 1, 0)
    ).astype(np.float32)
    # a8m: step-masked copy (off-step columns -5 -> exp(10x) ~ 2e-22)
    a8m = np.full((128, 2, NSTEP, NSTEP), -5.0, dtype=np.float32)
    for i in range(NSTEP):
        a8m[:, :, i, i] = a8[:, :, i]
    # b8[p, bh, i] = b_emb[i, 128 bh + p]
    b8 = np.ascontiguousarray(
        b_emb.reshape(NSTEP, 2, 128).transpose(2, 1, 0)
    ).astype(np.float32)
    tri = (p[:, None] >= p[None, :] + 1).astype(np.float32)
    msk = (np.arange(8)[None, :] // 2 == np.arange(NSTEP)[:, None]).astype(
        np.float32
    )
    return {"bwin": bwin, "a8": a8, "a8m": a8m, "b8": b8, "tri": tri,
            "msk": msk}


def kernel(a_emb, b_emb, W1, W2_sum, W2_carry):
    global LAST_EXEC_TIME_NS
    a_emb = np.asarray(a_emb, dtype=np.float32)
    b_emb = np.asarray(b_emb, dtype=np.float32)
    W1 = np.asarray(W1, dtype=np.float32)
    W2_sum = np.asarray(W2_sum, dtype=np.float32)
    W2_carry = np.asarray(W2_carry, dtype=np.float32)

    if not _structure_ok(W1, W2_sum, W2_carry):
        return _numpy_fallback(a_emb, b_emb, W1, W2_sum, W2_carry)

    from concourse.bass_utils import run_bass_kernel_spmd

    if "nc" not in _CACHE:
        _CACHE["nc"] = _build_nc()
    nc = _CACHE["nc"]

    in_map = _prep_inputs(a_emb, b_emb)
    trace = os.environ.get("KERNEL_TRACE", "") == "1"
    res = run_bass_kernel_spmd(nc, [in_map], [0], trace=trace)
    LAST_EXEC_TIME_NS = res.exec_time_ns
    return np.asarray(res.results[0]["out"], dtype=np.float32)
